# revision 14
# baseline (speedup 1.0000x reference)
"""CBOW negative-sampling loss kernel for 8 TRN2 NeuronCores.

Strategy (data-parallel, per sharding hint):
  - Shard the batch (B=16384) across 8 cores -> 2048 rows/core.
  - Replicate both embedding tables in each core's DRAM.
  - Per core, a 3-pass gather gets around two HW limits: the qPoolDynamic
    indirect DMA moves only 128 rows per ~1us GpSimd instruction
    (which would be GpSimd-bound ~6x off the memory roofline), and the
    batched dma_gather ucode takes int16 indices (<32768) while
    VOCAB=100000:
      pass 1: host groups each core's 43008 lookups by vocab chunk of
              32768 rows; one big dma_gather per chunk (int16-rebased
              indices, 0-padded to a fixed capacity) pulls the rows into
              SBUF in grouped order;
      pass 2: contiguous DMA dumps them to a DRAM scratch (<32768 rows
              per half, so scratch positions fit int16);
      pass 3: dma_gather from the scratch by host-computed positions
              regathers rows into compute order.
  - DVE computes the per-row loss terms, ACT the log-sigmoids; partial
    sums are DMA'd out and the final scalar reduction happens on host.
"""

import numpy as np

import concourse.bacc as bacc
import concourse.bass as bass
import concourse.mybir as mybir
import concourse.tile as tile
from concourse.bass_utils import run_bass_kernel_spmd

VOCAB = 100000
DIM = 128
B = 16384
CWIN = 10
K = 10
EPS = 1e-9
NCORES = 8
P = 128
BPC = B // NCORES            # 2048 batch rows per core
NTILES = BPC // P            # 16 tiles of 128 rows
CHUNK = 4                    # batch-tiles per regather chunk
NCHUNKS = NTILES // CHUNK
NIDX = CWIN + 1 + K          # 21 lookups per batch row

VCHUNK = 32768               # vocab rows per int16-addressable chunk
NVC = 4                      # number of vocab chunks (last one is 1696 rows)
GPIECE = 1024                # max indices per dma_gather (the 16KB SWDGE
                             # descriptor ring overflows above ~1024 descs)
# fixed per-(half, vocab-chunk) gather capacities, multiples of GPIECE,
# >=7 sigma above the binomial means of the chunk occupancies
CTX_CAPS = [7168, 7168, 7168, 1024]
TN_CAPS = [8192, 8192, 8192, 1024]
CTX_N = NTILES * CWIN * P    # 20480 context lookups per core
TN_N = NTILES * (K + 1) * P  # 22528 target+negative lookups per core
SCR_CTX_ROWS = sum(CTX_CAPS)  # 22016 (< 32768 so positions fit int16)
SCR_TN_ROWS = sum(TN_CAPS)    # 24320

# int16 index-tensor column layout (16-wrapped: 16 cols per 256 indices)
CTX_CH_COLS = [c // 16 for c in CTX_CAPS]
TN_CH_COLS = [c // 16 for c in TN_CAPS]
R_CTX_COLS = CTX_N // 16     # 1280
R_TN_COLS = TN_N // 16       # 1408
IDX_COLS = sum(CTX_CH_COLS) + sum(TN_CH_COLS) + R_CTX_COLS + R_TN_COLS

F32 = mybir.dt.float32
I16 = mybir.dt.int16
MULT = mybir.AluOpType.mult
ADD = mybir.AluOpType.add
AX_X = mybir.AxisListType.X
SIGMOID = mybir.ActivationFunctionType.Sigmoid
LN = mybir.ActivationFunctionType.Ln


def _scr_dump_ap(scr_ap, base_row, nrows):
    """DRAM scratch AP that places gathered slot j (partition j%128, block
    j//128) at scratch row base_row+j: dims (p, block, d) with steps
    (DIM, 128*DIM, 1)."""
    nb = nrows // P
    return bass.AP(
        scr_ap.tensor,
        base_row * DIM,
        [[DIM, P], [P * DIM, nb], [1, DIM]],
    )


def build_kernel_body(tc, idx, in_emb, out_emb, scr_ctx, scr_tn, usum):
    nc = tc.nc
    with (
        tc.tile_pool(name="io", bufs=1) as io_pool,
        tc.tile_pool(name="ph1", bufs=2) as ph1_pool,
        tc.tile_pool(name="gather", bufs=2) as gpool,
        tc.tile_pool(name="work", bufs=2) as wpool,
    ):
        idx_t = io_pool.tile([P, IDX_COLS], I16)
        nc.sync.dma_start(out=idx_t[:], in_=idx[:, :])

        eps_t = io_pool.tile([P, 1], F32)
        nc.vector.memset(eps_t[:], EPS)

        us = io_pool.tile([P, NTILES], F32)

        # ---- pass 1+2: chunk-grouped gather, dump to scratch ----
        # one GPIECE-sized gather + dump per piece (descriptor-ring limit)
        col = 0
        for half, (table, caps, scr) in enumerate(
            [(in_emb, CTX_CAPS, scr_ctx), (out_emb, TN_CAPS, scr_tn)]
        ):
            base_row = 0
            for c in range(NVC):
                cap = caps[c]
                rows_c = min(VCHUNK, VOCAB - c * VCHUNK)
                for i in range(cap // GPIECE):
                    g = ph1_pool.tile([P, (GPIECE // P) * DIM], F32, tag="ph1")
                    g3 = g[:].rearrange("p (b d) -> p b d", d=DIM)
                    nc.gpsimd.dma_gather(
                        out_ap=g3,
                        in_ap=table[c * VCHUNK : c * VCHUNK + rows_c, :],
                        idxs_ap=idx_t[
                            :, col + i * (GPIECE // 16) :
                            col + (i + 1) * (GPIECE // 16)
                        ],
                        num_idxs=GPIECE,
                        num_idxs_reg=GPIECE,
                        elem_size=DIM,
                        queue_num=0,
                    )
                    nc.sync.dma_start(
                        out=_scr_dump_ap(scr, base_row + i * GPIECE, GPIECE),
                        in_=g[:],
                    )
                col += cap // 16
                base_row += cap
        r_ctx_col = col
        r_tn_col = col + R_CTX_COLS

        # ---- pass 3 + compute ----
        for cc in range(NCHUNKS):
            ctx_g = gpool.tile([P, CHUNK * CWIN * DIM], F32, tag="ctx")
            tn_g = gpool.tile([P, CHUNK * (K + 1) * DIM], F32, tag="tn")
            n_ctx = CHUNK * CWIN * P          # 5120
            n_tn = CHUNK * (K + 1) * P        # 5632
            for scr, g_t, n_all, rcol in (
                (scr_ctx, ctx_g, n_ctx, r_ctx_col),
                (scr_tn, tn_g, n_tn, r_tn_col),
            ):
                g3 = g_t[:].rearrange("p (b d) -> p b d", d=DIM)
                base = rcol + cc * (n_all // 16)
                off = 0
                while off < n_all:
                    n_p = min(GPIECE, n_all - off)
                    nc.gpsimd.dma_gather(
                        out_ap=g3[:, off // P : (off + n_p) // P, :],
                        in_ap=scr[:, :],
                        idxs_ap=idx_t[
                            :, base + off // 16 : base + (off + n_p) // 16
                        ],
                        num_idxs=n_p,
                        num_idxs_reg=n_p,
                        elem_size=DIM,
                        queue_num=0,
                    )
                    off += n_p

            for b in range(CHUNK):
                t_idx = cc * CHUNK + b
                bc = b * CWIN * DIM        # base into ctx_g
                bt = b * (K + 1) * DIM     # base into tn_g

                # context sum over the 10 window rows (tree of adds)
                a1 = wpool.tile([P, 5 * DIM], F32, tag="a1")
                nc.vector.tensor_add(
                    a1[:], ctx_g[:, bc : bc + 5 * DIM],
                    ctx_g[:, bc + 5 * DIM : bc + 10 * DIM],
                )
                b1 = wpool.tile([P, 2 * DIM], F32, tag="b1")
                nc.vector.tensor_add(
                    b1[:], a1[:, 0 : 2 * DIM], a1[:, 2 * DIM : 4 * DIM]
                )
                csum = wpool.tile([P, DIM], F32, tag="csum")
                nc.vector.tensor_add(csum[:], b1[:, 0:DIM], b1[:, DIM : 2 * DIM])
                nc.vector.tensor_add(csum[:], csum[:], a1[:, 4 * DIM : 5 * DIM])

                # scores: s[:,0] = sum_d csum*tgt ; s[:,1+k] = sum_d csum*neg_k
                s = wpool.tile([P, 1 + K], F32, tag="s")
                prod = wpool.tile([P, (K + 1) * DIM], F32, tag="prod")
                prod3 = prod[:].rearrange("p (k d) -> p k d", d=DIM)
                tn3 = tn_g[:, bt : bt + (K + 1) * DIM].rearrange(
                    "p (k d) -> p k d", d=DIM
                )
                csum_b = csum[:][:, None, :].to_broadcast([P, K + 1, DIM])
                nc.vector.tensor_tensor(prod3, tn3, csum_b, MULT)
                nc.vector.tensor_reduce(
                    out=s[:, 0 : 1 + K], in_=prod3, axis=AX_X, op=ADD
                )
                # flip the target column so sigmoid(-0.1*s) = sigmoid(+pos)
                nc.vector.tensor_scalar_mul(s[:, 0:1], s[:, 0:1], -1.0)

                # loss terms; the /10 context-mean is folded into the
                # activation scale
                sig = wpool.tile([P, 1 + K], F32, tag="sig")
                nc.scalar.activation(sig[:], s[:], SIGMOID, scale=-0.1)
                lnv = wpool.tile([P, 1 + K], F32, tag="lnv")
                nc.scalar.activation(
                    lnv[:], sig[:], LN, bias=eps_t[:],
                    accum_out=us[:, t_idx : t_idx + 1],
                )

        nc.sync.dma_start(out=usum[:, :], in_=us[:])


def build_nc():
    nc = bacc.Bacc(
        "TRN2",
        target_bir_lowering=False,
        debug=False,
        enable_asserts=False,
        num_devices=NCORES,
    )
    idx = nc.dram_tensor("idx", [P, IDX_COLS], I16, kind="ExternalInput")
    in_emb = nc.dram_tensor("in_emb", [VOCAB, DIM], F32, kind="ExternalInput")
    out_emb = nc.dram_tensor("out_emb", [VOCAB, DIM], F32, kind="ExternalInput")
    scr_ctx = nc.dram_tensor("scr_ctx", [SCR_CTX_ROWS, DIM], F32)
    scr_tn = nc.dram_tensor("scr_tn", [SCR_TN_ROWS, DIM], F32)
    usum = nc.dram_tensor("usum", [P, NTILES], F32, kind="ExternalOutput")
    with tile.TileContext(nc) as tc:
        build_kernel_body(
            tc, idx.ap(), in_emb.ap(), out_emb.ap(),
            scr_ctx.ap(), scr_tn.ap(), usum.ap(),
        )
    nc.compile()
    return nc


def _wrap16(arr):
    """[n] int16 -> [128, n/16] SBUF layout: index j at (j%16, j//16),
    replicated to all 8 groups of 16 partitions (Q7 core pairs read their
    own group)."""
    w = arr.reshape(-1, 16).T  # [16, n/16]
    return np.tile(w, (8, 1))


def _sort_half(stream):
    """Group a lookup stream by vocab chunk.

    Returns (chunk_lists, positions): chunk_lists[c] is the 0-padded int16
    local-index list for chunk c (length caps[c]); positions[e] is the
    scratch row where stream entry e lands."""
    chunks = stream // VCHUNK
    pos = np.empty(stream.shape[0], dtype=np.int64)
    out_lists = []
    base = 0
    caps = CTX_CAPS if stream.shape[0] == CTX_N else TN_CAPS
    for c in range(NVC):
        sel = np.nonzero(chunks == c)[0]
        n_c = sel.shape[0]
        cap = caps[c]
        assert n_c <= cap, f"chunk {c} count {n_c} exceeds cap {cap}"
        lst = np.zeros(cap, dtype=np.int16)
        lst[:n_c] = (stream[sel] - c * VCHUNK).astype(np.int16)
        out_lists.append(lst)
        pos[sel] = base + np.arange(n_c)
        base += cap
    return out_lists, pos


def make_in_maps(context, target, negatives, in_emb, out_emb):
    context = np.asarray(context).astype(np.int64)
    target = np.asarray(target).astype(np.int64)
    negatives = np.asarray(negatives).astype(np.int64)
    in_emb = np.ascontiguousarray(np.asarray(in_emb, dtype=np.float32))
    out_emb = np.ascontiguousarray(np.asarray(out_emb, dtype=np.float32))
    tn_full = np.concatenate([target[:, None], negatives], axis=1)  # [B, 11]
    in_maps = []
    for c in range(NCORES):
        ctx_sl = context[c * BPC : (c + 1) * BPC]  # [2048, 10]
        tn_sl = tn_full[c * BPC : (c + 1) * BPC]   # [2048, 11]
        # compute-order streams: entry e = slot*128 + p,
        # slot = tile*CWIN + j (ctx) or tile*(K+1) + j (tn)
        ctx_stream = (
            ctx_sl.reshape(NTILES, P, CWIN).transpose(0, 2, 1).reshape(-1)
        )
        tn_stream = (
            tn_sl.reshape(NTILES, P, K + 1).transpose(0, 2, 1).reshape(-1)
        )
        ctx_lists, ctx_pos = _sort_half(ctx_stream)
        tn_lists, tn_pos = _sort_half(tn_stream)
        cols = [_wrap16(l) for l in ctx_lists]
        cols += [_wrap16(l) for l in tn_lists]
        cols.append(_wrap16(ctx_pos.astype(np.int16)))
        cols.append(_wrap16(tn_pos.astype(np.int16)))
        idx16 = np.ascontiguousarray(np.concatenate(cols, axis=1))
        assert idx16.shape == (P, IDX_COLS)
        in_maps.append(
            {"idx": idx16, "in_emb": in_emb, "out_emb": out_emb}
        )
    return in_maps


_NC_CACHE = []
LAST_RESULT = None  # BassKernelResults of the most recent run (for profiling)


def kernel(**inputs) -> np.ndarray:
    global LAST_RESULT
    in_maps = make_in_maps(
        inputs["context"],
        inputs["target"],
        inputs["negatives"],
        inputs["in_emb"],
        inputs["out_emb"],
    )
    if not _NC_CACHE:
        _NC_CACHE.append(build_nc())
    nc = _NC_CACHE[0]
    res = run_bass_kernel_spmd(nc, in_maps, core_ids=list(range(NCORES)))
    LAST_RESULT = res
    total = sum(float(r["usum"].astype(np.float64).sum()) for r in res.results)
    return np.array(-total / B, dtype=np.float32)


# revision 15
# speedup vs baseline: 2.3353x; 2.3353x over previous
"""CBOW negative-sampling loss kernel for 8 TRN2 NeuronCores.

Strategy (data-parallel, per sharding hint):
  - Shard the batch (B=16384) across 8 cores -> 2048 rows/core.
  - Replicate both embedding tables in each core's DRAM.
  - Per core: 336 indirect DMAs gather the 43008 embedding rows
    (the qPoolDynamic ucode consumes one index per partition per
    instruction, i.e. 128 rows / 64KB per ~1us GpSimd instruction;
    the batched dma_gather ucode was measured ~20 GB/s/core on this
    access pattern, so per-128-row indirect gathers are the fastest
    available primitive).  DVE computes the per-row loss terms, ACT
    the log-sigmoids; per-tile partial sums are DMA'd out and the
    final scalar reduction happens on host.
"""

import numpy as np

import concourse.bacc as bacc
import concourse.bass as bass
import concourse.mybir as mybir
import concourse.tile as tile
from concourse.bass_utils import run_bass_kernel_spmd

VOCAB = 100000
DIM = 128
B = 16384
CWIN = 10
K = 10
EPS = 1e-9
NCORES = 8
P = 128
BPC = B // NCORES            # 2048 batch rows per core
NTILES = BPC // P            # 16 tiles of 128 rows
CHUNK = 4                    # batch-tiles per gather chunk
NCHUNKS = NTILES // CHUNK
NIDX = CWIN + 1 + K          # 21 lookups per batch row

F32 = mybir.dt.float32
MULT = mybir.AluOpType.mult
ADD = mybir.AluOpType.add
AX_X = mybir.AxisListType.X
SIGMOID = mybir.ActivationFunctionType.Sigmoid
LN = mybir.ActivationFunctionType.Ln

GATHER_BUFS = 2


def build_kernel_body(tc, idx, in_emb, out_emb, usum):
    """Emit the per-core program.

    idx:    [P, NTILES*NIDX] int32 SBUF-layout indices.  Cols 0..159 are
            context lookups (free pos t*10+j -> in_emb row for batch row
            t*128+partition, window slot j); cols 160..335 are target+neg
            lookups (free pos 160 + t*11 + j -> out_emb row; j=0 target,
            j=1..10 negatives).
    usum:   [P, NTILES] f32; column t = per-row sum of
            log(sigmoid(pos)+eps) + sum_k log(sigmoid(-neg_k)+eps).
    """
    nc = tc.nc
    ctx_cols = NTILES * CWIN          # 160
    with (
        tc.tile_pool(name="io", bufs=1) as io_pool,
        tc.tile_pool(name="gather", bufs=GATHER_BUFS) as gpool,
        tc.tile_pool(name="work", bufs=2) as wpool,
    ):
        idx_t = io_pool.tile([P, NTILES * NIDX], mybir.dt.int32)
        nc.sync.dma_start(out=idx_t[:], in_=idx[:, :])

        eps_t = io_pool.tile([P, 1], F32)
        nc.vector.memset(eps_t[:], EPS)

        us = io_pool.tile([P, NTILES], F32)

        for c in range(NCHUNKS):
            ctx_g = gpool.tile([P, CHUNK * CWIN * DIM], F32, tag="ctx")
            tn_g = gpool.tile([P, CHUNK * (K + 1) * DIM], F32, tag="tn")
            c0 = c * CHUNK * CWIN
            t0 = ctx_cols + c * CHUNK * (K + 1)
            # one 128-row gather per (tile, lookup) column
            for q in range(CHUNK * CWIN):
                nc.gpsimd.indirect_dma_start(
                    out=ctx_g[:, q * DIM : (q + 1) * DIM],
                    out_offset=None,
                    in_=in_emb[:, :],
                    in_offset=bass.IndirectOffsetOnAxis(
                        ap=idx_t[:, c0 + q : c0 + q + 1], axis=0
                    ),
                )
            for q in range(CHUNK * (K + 1)):
                nc.gpsimd.indirect_dma_start(
                    out=tn_g[:, q * DIM : (q + 1) * DIM],
                    out_offset=None,
                    in_=out_emb[:, :],
                    in_offset=bass.IndirectOffsetOnAxis(
                        ap=idx_t[:, t0 + q : t0 + q + 1], axis=0
                    ),
                )

            for b in range(CHUNK):
                t_idx = c * CHUNK + b
                bc = b * CWIN * DIM        # base into ctx_g
                bt = b * (K + 1) * DIM     # base into tn_g

                # context sum over the 10 window rows (tree of adds)
                a1 = wpool.tile([P, 5 * DIM], F32, tag="a1")
                nc.vector.tensor_add(
                    a1[:], ctx_g[:, bc : bc + 5 * DIM],
                    ctx_g[:, bc + 5 * DIM : bc + 10 * DIM],
                )
                b1 = wpool.tile([P, 2 * DIM], F32, tag="b1")
                nc.vector.tensor_add(
                    b1[:], a1[:, 0 : 2 * DIM], a1[:, 2 * DIM : 4 * DIM]
                )
                csum = wpool.tile([P, DIM], F32, tag="csum")
                nc.vector.tensor_add(csum[:], b1[:, 0:DIM], b1[:, DIM : 2 * DIM])
                nc.vector.tensor_add(csum[:], csum[:], a1[:, 4 * DIM : 5 * DIM])

                # scores: s[:,0] = sum_d csum*tgt ; s[:,1+k] = sum_d csum*neg_k
                s = wpool.tile([P, 1 + K], F32, tag="s")
                prod = wpool.tile([P, (K + 1) * DIM], F32, tag="prod")
                prod3 = prod[:].rearrange("p (k d) -> p k d", d=DIM)
                tn3 = tn_g[:, bt : bt + (K + 1) * DIM].rearrange(
                    "p (k d) -> p k d", d=DIM
                )
                csum_b = csum[:][:, None, :].to_broadcast([P, K + 1, DIM])
                nc.vector.tensor_tensor(prod3, tn3, csum_b, MULT)
                nc.vector.tensor_reduce(
                    out=s[:, 0 : 1 + K], in_=prod3, axis=AX_X, op=ADD
                )
                # flip the target column so sigmoid(-0.1*s) = sigmoid(+pos)
                nc.vector.tensor_scalar_mul(s[:, 0:1], s[:, 0:1], -1.0)

                # loss terms; the /10 context-mean is folded into the
                # activation scale
                sig = wpool.tile([P, 1 + K], F32, tag="sig")
                nc.scalar.activation(sig[:], s[:], SIGMOID, scale=-0.1)
                lnv = wpool.tile([P, 1 + K], F32, tag="lnv")
                nc.scalar.activation(
                    lnv[:], sig[:], LN, bias=eps_t[:],
                    accum_out=us[:, t_idx : t_idx + 1],
                )

        nc.sync.dma_start(out=usum[:, :], in_=us[:])


def build_nc():
    nc = bacc.Bacc(
        "TRN2",
        target_bir_lowering=False,
        debug=False,
        enable_asserts=False,
        num_devices=NCORES,
    )
    idx = nc.dram_tensor(
        "idx", [P, NTILES * NIDX], mybir.dt.int32, kind="ExternalInput"
    )
    in_emb = nc.dram_tensor("in_emb", [VOCAB, DIM], F32, kind="ExternalInput")
    out_emb = nc.dram_tensor("out_emb", [VOCAB, DIM], F32, kind="ExternalInput")
    usum = nc.dram_tensor("usum", [P, NTILES], F32, kind="ExternalOutput")
    with tile.TileContext(nc) as tc:
        build_kernel_body(tc, idx.ap(), in_emb.ap(), out_emb.ap(), usum.ap())
    nc.compile()
    return nc


def _wrap16(arr):
    """[n] int16 -> [128, n/16] SBUF layout for dma_gather index lists
    (kept for the experiment scripts)."""
    w = arr.reshape(-1, 16).T
    return np.tile(w, (8, 1))


def make_in_maps(context, target, negatives, in_emb, out_emb):
    context = np.asarray(context).astype(np.int32)
    target = np.asarray(target).astype(np.int32)
    negatives = np.asarray(negatives).astype(np.int32)
    in_emb = np.ascontiguousarray(np.asarray(in_emb, dtype=np.float32))
    out_emb = np.ascontiguousarray(np.asarray(out_emb, dtype=np.float32))
    tn_full = np.concatenate([target[:, None], negatives], axis=1)  # [B, 11]
    in_maps = []
    for c in range(NCORES):
        ctx_sl = context[c * BPC : (c + 1) * BPC]  # [2048, 10]
        tn_sl = tn_full[c * BPC : (c + 1) * BPC]   # [2048, 11]
        ctx_tiles = (
            ctx_sl.reshape(NTILES, P, CWIN)
            .transpose(1, 0, 2)
            .reshape(P, NTILES * CWIN)
        )
        tn_tiles = (
            tn_sl.reshape(NTILES, P, K + 1)
            .transpose(1, 0, 2)
            .reshape(P, NTILES * (K + 1))
        )
        tiles = np.concatenate([ctx_tiles, tn_tiles], axis=1)  # [P, 336]
        in_maps.append(
            {
                "idx": np.ascontiguousarray(tiles),
                "in_emb": in_emb,
                "out_emb": out_emb,
            }
        )
    return in_maps


_NC_CACHE = []
LAST_RESULT = None  # BassKernelResults of the most recent run (for profiling)


def kernel(**inputs) -> np.ndarray:
    global LAST_RESULT
    in_maps = make_in_maps(
        inputs["context"],
        inputs["target"],
        inputs["negatives"],
        inputs["in_emb"],
        inputs["out_emb"],
    )
    if not _NC_CACHE:
        _NC_CACHE.append(build_nc())
    nc = _NC_CACHE[0]
    res = run_bass_kernel_spmd(nc, in_maps, core_ids=list(range(NCORES)))
    LAST_RESULT = res
    total = sum(float(r["usum"].astype(np.float64).sum()) for r in res.results)
    return np.array(-total / B, dtype=np.float32)
